# revision 2
# baseline (speedup 1.0000x reference)
"""BiMPNN layer on 8 Trainium2 NeuronCores — v2.

Math (reassociated): out = gelu( (A h) W^T + (A^T h) Wt^T + h Ws^T
                                 + deg_out*W_b + deg_in*Wt_b + Ws_b )

Distribution: natural node order — core c owns rows [c*S, (c+1)*S).
h enters sharded (shard_map splits the full array), is cast to bf16 and
AllGathered on-device into a full gather table, so the host->device h
traffic is 1/8 of the replicated baseline.  Static edge metadata (gather
indices, dest-slot arrays, degrees) is cached on device across calls.

Per core: destinations processed in groups of 512 slots (one PSUM bank).
For each (group, allgather-chunk, direction): one dma_gather pulls the
needed source rows (bf16, 256B each); per 128-edge chunk a DVE is_equal
builds a one-hot [128, 512] selector and the PE accumulates
psum[feat, dest] += ged_chunk^T @ sel.  The dense stage consumes the
transposed aggregates directly (lhsT = G^T block), so the output comes
out node-major [S, 128] with zero host-side unpermute.  The self term's
h^T block is loaded with an HWDGE DMA-transpose from the bf16 bounce
buffer (no gather, no PE transpose).
"""

import hashlib
import json

import numpy as np
import ml_dtypes

import concourse.bass as bass
import concourse.mybir as mybir
import concourse.tile as tile
import concourse.bass_utils as bass_utils
import concourse.bass2jax as bass2jax
from concourse import library_config
from concourse.tile_rust import add_dep_helper

# ---------------------------------------------------------------------------
# BIR fixup (this walrus build lowers at most ONE sync wait per instruction):
# hoist excess waits onto same-engine NoOps placed just before the
# offending instruction.
_MAX_WAITS = 1


def _split_excess_waits(bir_json: bytes) -> bytes:
    m = json.loads(bir_json)
    ctr = 0
    changed = False
    for fn in m["functions"]:
        for blk in fn["blocks"]:
            new_insts = []
            for inst in blk["instructions"]:
                body = inst
                if len(inst) == 1 and isinstance(next(iter(inst.values())), dict):
                    body = inst[next(iter(inst))]
                si = body.get("sync_info") if isinstance(body, dict) else None
                waits = si.get("on_wait") if si else None
                if waits and len(waits) > _MAX_WAITS:
                    changed = True
                    excess, keep = waits[:-_MAX_WAITS], waits[-_MAX_WAITS:]
                    while excess:
                        part, excess = excess[:_MAX_WAITS], excess[_MAX_WAITS:]
                        ctr += 1
                        new_insts.append({
                            "debug": body.get("debug", 0),
                            "engine": body.get("engine"),
                            "ins": [], "outs": [],
                            "name": f"I-waitsplit-{ctr}",
                            "opcode": "NoOp",
                            "sync_info": {"on_update": [], "on_wait": part},
                        })
                    si["on_wait"] = keep
                new_insts.append(inst)
            blk["instructions"] = new_insts
    if changed:
        return json.dumps(m).encode()
    return bir_json


if not getattr(bass_utils, "_waitsplit_patched", False):
    _orig_compile_bir_kernel = bass_utils.compile_bir_kernel

    def _patched_compile_bir_kernel(bir_json, tmpdir, neff_name="file.neff"):
        return _orig_compile_bir_kernel(
            _split_excess_waits(bir_json), tmpdir, neff_name)

    bass_utils.compile_bir_kernel = _patched_compile_bir_kernel
    bass2jax.compile_bir_kernel = _patched_compile_bir_kernel
    bass_utils._waitsplit_patched = True

# ---------------------------------------------------------------------------

F32 = mybir.dt.float32
BF16 = mybir.dt.bfloat16
I16 = mybir.dt.int16

FULL_CFG = dict(N=100000, D=128, NCORES=8, GROUP=512, NCHUNK=4)


def _cdiv(a, b):
    return -(-a // b)


def _derive(cfg):
    N, D, NC, GROUP, NCHUNK = (cfg["N"], cfg["D"], cfg["NCORES"],
                               cfg["GROUP"], cfg["NCHUNK"])
    S = N // NC
    assert S * NC == N
    CH = S // NCHUNK
    assert CH * NCHUNK == S
    BUCKET = NC * CH
    assert BUCKET <= 32768
    NG = _cdiv(S, GROUP)
    SP = _cdiv(S, 256) * 256          # bounce rows, padded for DMA-transpose
    return S, CH, BUCKET, NG, SP


# ---------------------------------------------------------------------------
# Host-side preprocessing (static per graph)
def _pack_dir(dest, src, cfg):
    N = cfg["N"]
    NC, GROUP, NCHUNK = cfg["NCORES"], cfg["GROUP"], cfg["NCHUNK"]
    S, CH, BUCKET, NG, _ = _derive(cfg)
    E = dest.shape[0]

    cd = dest // S
    dloc = dest % S
    g = dloc // GROUP
    # dest-slot id encoded as a bf16 BIT PATTERN (0x4000 + slot): distinct
    # normal values for slots 0..511, so an all-bf16 is_equal against the
    # same-encoded iota is exact.  Pad value 0x0000 (+0.0) matches nothing.
    slot = (0x4000 + (dloc % GROUP)).astype(np.uint16)
    cs = src // S
    sloc = src % S
    k = sloc // CH
    bidx = (cs * CH + sloc % CH).astype(np.int16)

    key = (cd * NG + g) * NCHUNK + k
    o = np.argsort(key, kind="stable")
    ks = key[o]
    cnt = np.bincount(ks, minlength=NC * NG * NCHUNK)
    KB = np.ceil(cnt.reshape(NC * NG, NCHUNK) / 128).astype(int).max(axis=0)
    OFF = np.concatenate([[0], np.cumsum(KB)[:-1]]).astype(int)
    KBs = int(KB.sum())

    start = np.concatenate([[0], np.cumsum(cnt)[:-1]])
    r = np.arange(E) - start[ks]

    gx = np.zeros((NC, 128, NG * KBs * 8), np.int16)
    dl = np.zeros((NC, 128, NG * KBs), np.uint16)

    core = ks // (NG * NCHUNK)
    gg = (ks // NCHUNK) % NG
    kk = ks % NCHUNK
    colseg = gg * KBs + OFF[kk]
    chunk = r // 128
    p = r % 128
    dl[core, p, colseg + chunk] = slot[o]
    dl = dl.view(ml_dtypes.bfloat16)
    col = colseg * 8 + r // 16
    prt = r % 16
    gx4 = gx.reshape(NC, 8, 16, NG * KBs * 8)
    gx4[core, :, prt, col] = bidx[o][:, None]
    return tuple(int(x) for x in KB), gx, dl


def pack_graph(rows, cols, cfg):
    N, NC, GROUP = cfg["N"], cfg["NCORES"], cfg["GROUP"]
    S, CH, BUCKET, NG, SP = _derive(cfg)
    rows = np.asarray(rows).astype(np.int64)
    cols = np.asarray(cols).astype(np.int64)

    KB1, gx1, dl1 = _pack_dir(rows, cols, cfg)
    KB2, gx2, dl2 = _pack_dir(cols, rows, cfg)

    deg1 = np.bincount(rows, minlength=N).astype(np.float32)
    deg2 = np.bincount(cols, minlength=N).astype(np.float32)
    degs = np.zeros((NC, 3, NG * GROUP), np.float32)
    for c in range(NC):
        sl = np.arange(c * S, (c + 1) * S)
        degs[c, 0, :S] = deg1[sl]
        degs[c, 1, :S] = deg2[sl]
        degs[c, 2, :S] = 1.0
    return dict(KB1=KB1, KB2=KB2, gx1=gx1, dl1=dl1, gx2=gx2, dl2=dl2,
                degs=degs.astype(ml_dtypes.bfloat16))


# ---------------------------------------------------------------------------
# Device program
def build_nc(cfg, KB1, KB2):
    N, D, NC, GROUP, NCHUNK = (cfg["N"], cfg["D"], cfg["NCORES"],
                               cfg["GROUP"], cfg["NCHUNK"])
    S, CH, BUCKET, NG, SP = _derive(cfg)
    OFF1 = np.concatenate([[0], np.cumsum(KB1)[:-1]]).astype(int)
    OFF2 = np.concatenate([[0], np.cumsum(KB2)[:-1]]).astype(int)
    KB1s, KB2s = int(sum(KB1)), int(sum(KB2))

    nc = bass.Bass(num_devices=NC)
    h_in = nc.declare_dram_parameter("h", [S, D], F32, isOutput=False)
    wT3 = nc.declare_dram_parameter("wT3", [D, 3 * D], BF16, isOutput=False)
    b3 = nc.declare_dram_parameter("b3", [3, D], BF16, isOutput=False)
    iotam = nc.declare_dram_parameter("iotam", [128, GROUP], BF16,
                                      isOutput=False)
    gx1 = nc.declare_dram_parameter("gx1", [128, NG * KB1s * 8], I16,
                                    isOutput=False)
    dl1 = nc.declare_dram_parameter("dl1", [128, NG * KB1s], BF16,
                                    isOutput=False)
    gx2 = nc.declare_dram_parameter("gx2", [128, NG * KB2s * 8], I16,
                                    isOutput=False)
    dl2 = nc.declare_dram_parameter("dl2", [128, NG * KB2s], BF16,
                                    isOutput=False)
    degs = nc.declare_dram_parameter("degs", [3, NG * GROUP], BF16,
                                     isOutput=False)
    out = nc.declare_dram_parameter("out", [S, D], F32, isOutput=True)

    bounce = nc.dram_tensor("bounce", [SP, D], BF16)
    table = nc.dram_tensor("table", [N, D], BF16)

    with tile.TileContext(nc) as tc:
        with (
            tc.tile_pool(name="const", bufs=1) as cpool,
            tc.tile_pool(name="aux", bufs=2) as apool,
            tc.tile_pool(name="ged", bufs=2) as gpool,
            tc.tile_pool(name="sel", bufs=4) as spool,
            tc.tile_pool(name="work", bufs=2) as wpool,
            tc.tile_pool(name="outp", bufs=4) as opool,
            tc.tile_pool(name="psum", bufs=2, space="PSUM") as ppool,
            tc.tile_pool(name="psod", bufs=4, space="PSUM") as ppod,
        ):
            lib = nc.gpsimd.load_library(library_config.mlp)
            _regs = {}

            def nreg(v):
                if v not in _regs:
                    _regs[v] = nc.gpsimd.to_reg(v)
                return _regs[v]

            iota_sb = cpool.tile([128, GROUP], BF16)
            nc.sync.dma_start(out=iota_sb[:], in_=iotam[:])
            wT3_sb = cpool.tile([D, 3 * D], BF16)
            nc.sync.dma_start(out=wT3_sb[:], in_=wT3[:])
            b3_sb = cpool.tile([3, D], BF16)
            nc.sync.dma_start(out=b3_sb[:], in_=b3[:])
            degs_sb = cpool.tile([3, NG * GROUP], BF16)
            nc.sync.dma_start(out=degs_sb[:], in_=degs[:])

            # h: cast shard to bf16, then AllGather chunks into the table
            for k in range(NCHUNK):
                cast = nc.gpsimd.dma_start(
                    out=bounce[k * CH:(k + 1) * CH, :],
                    in_=h_in[k * CH:(k + 1) * CH, :])
                add_dep_helper(cast.ins, lib.ins, False, "lib first")
                nc.gpsimd.collective_compute(
                    "AllGather", mybir.AluOpType.bypass,
                    replica_groups=[list(range(NC))],
                    ins=[bounce[k * CH:(k + 1) * CH, :].opt()],
                    outs=[table[k * BUCKET:(k + 1) * BUCKET, :].opt()])

            dirs = ((KB1, OFF1, KB1s, gx1, dl1),
                    (KB2, OFF2, KB2s, gx2, dl2))

            for g in range(NG):
                gw = min(GROUP, S - g * GROUP)
                gwp = _cdiv(gw, 128) * 128

                hT = wpool.tile([D, GROUP], BF16, tag="hT")
                nc.sync.dma_start(out=hT[:, 0:gwp],
                                  in_=bounce[g * GROUP:g * GROUP + gwp, :],
                                  transpose=True)

                gTs = []
                for dn, (KB, OFF, KBs, gx, dl) in enumerate(dirs):
                    gx_sb = apool.tile([128, KBs * 8], I16, tag=f"gx{dn}")
                    nc.sync.dma_start(
                        out=gx_sb[:],
                        in_=gx[:, g * KBs * 8:(g + 1) * KBs * 8])
                    dl_sb = apool.tile([128, KBs], BF16, tag=f"dl{dn}")
                    nc.sync.dma_start(
                        out=dl_sb[:], in_=dl[:, g * KBs:(g + 1) * KBs])

                    ged = gpool.tile([128, KBs, 128], BF16, tag=f"ged{dn}")
                    for k in range(NCHUNK):
                        if KB[k] == 0:
                            continue
                        gi = nc.gpsimd.dma_gather(
                            out_ap=ged[:, OFF[k]:OFF[k] + KB[k], :],
                            in_ap=table[k * BUCKET:(k + 1) * BUCKET, :],
                            idxs_ap=gx_sb[:, OFF[k] * 8:(OFF[k] + KB[k]) * 8],
                            num_idxs=KB[k] * 128,
                            num_idxs_reg=nreg(KB[k] * 128),
                            elem_size=D, single_packet=False)
                        add_dep_helper(gi.ins, lib.ins, False, "lib first")

                    ps = ppool.tile([D, GROUP], F32, tag=f"ps{dn}")
                    for j in range(KBs):
                        sel = spool.tile([128, GROUP], BF16, tag="sel")
                        nc.vector.tensor_tensor(
                            out=sel[:],
                            in0=dl_sb[:, j:j + 1].to_broadcast([128, GROUP]),
                            in1=iota_sb[:],
                            op=mybir.AluOpType.is_equal)
                        nc.tensor.matmul(ps[:], lhsT=ged[:, j, :], rhs=sel[:],
                                         start=(j == 0), stop=(j == KBs - 1))
                    gT = wpool.tile([D, GROUP], BF16, tag=f"g{dn}T")
                    nc.vector.tensor_copy(gT[:], ps[:])
                    gTs.append(gT)

                g1T, g2T = gTs
                for b in range(_cdiv(gw, 128)):
                    w = min(128, gw - b * 128)
                    boff = b * 128
                    aoff = g * GROUP + boff
                    pso = ppod.tile([128, D], F32, tag="pso")
                    nc.tensor.matmul(pso[0:w, :], lhsT=g1T[:, boff:boff + w],
                                     rhs=wT3_sb[:, 0:D],
                                     start=True, stop=False)
                    nc.tensor.matmul(pso[0:w, :], lhsT=g2T[:, boff:boff + w],
                                     rhs=wT3_sb[:, D:2 * D],
                                     start=False, stop=False)
                    nc.tensor.matmul(pso[0:w, :], lhsT=hT[:, boff:boff + w],
                                     rhs=wT3_sb[:, 2 * D:3 * D],
                                     start=False, stop=False)
                    nc.tensor.matmul(pso[0:w, :],
                                     lhsT=degs_sb[:, aoff:aoff + w],
                                     rhs=b3_sb[:], start=False, stop=True)
                    ob = opool.tile([128, D], F32, tag="ob")
                    nc.scalar.activation(out=ob[0:w, :], in_=pso[0:w, :],
                                         func=mybir.ActivationFunctionType.Gelu)
                    nc.sync.dma_start(out=out[aoff:aoff + w, :],
                                      in_=ob[0:w, :])

    mybir.codegen_inst_isa_subclasses(nc)
    return nc


# ---------------------------------------------------------------------------
# Runner: persistent jit + device-cached statics
_SESS = {}


def _make_exec(nc, n_cores):
    import jax
    from jax.experimental.shard_map import shard_map
    from jax.sharding import Mesh, PartitionSpec

    bass2jax.install_neuronx_cc_hook()

    partition_name = (nc.partition_id_tensor.name
                      if nc.partition_id_tensor else None)
    in_names, out_names, out_avals = [], [], []
    for alloc in nc.m.functions[0].allocations:
        if not isinstance(alloc, mybir.MemoryLocationSet):
            continue
        name = alloc.memorylocations[0].name
        if alloc.kind == "ExternalInput":
            if name != partition_name:
                in_names.append(name)
        elif alloc.kind == "ExternalOutput":
            out_names.append(name)
            out_avals.append(jax.core.ShapedArray(
                tuple(alloc.tensor_shape), mybir.dt.np(alloc.dtype)))
    n_params = len(in_names)
    all_names = list(in_names) + list(out_names)
    if partition_name is not None:
        all_names_full = all_names + [partition_name]
    else:
        all_names_full = all_names

    assert nc.dbg_addr is None or not nc.dbg_callbacks

    def _body(*args):
        operands = list(args)
        if partition_name is not None:
            operands.append(bass2jax.partition_id_tensor())
        outs = bass2jax._bass_exec_p.bind(
            *operands,
            out_avals=tuple(out_avals),
            in_names=tuple(all_names_full),
            out_names=tuple(out_names),
            lowering_input_output_aliases=(),
            sim_require_finite=True,
            sim_require_nnan=True,
            nc=nc)
        return tuple(outs)

    devices = jax.devices()[:n_cores]
    mesh = Mesh(np.asarray(devices), ("core",))
    n_outs = len(out_names)
    donate = tuple(range(n_params, n_params + n_outs))
    fn = jax.jit(
        shard_map(_body, mesh=mesh,
                  in_specs=(PartitionSpec("core"),) * (n_params + n_outs),
                  out_specs=(PartitionSpec("core"),) * n_outs,
                  check_rep=False),
        donate_argnums=donate, keep_unused=True)
    return fn, in_names, out_names, out_avals, mesh


def _get_session(rows, cols, cfg):
    import jax
    from jax.sharding import NamedSharding, PartitionSpec

    key = (hashlib.md5(
        np.ascontiguousarray(rows).tobytes()
        + np.ascontiguousarray(cols).tobytes()).hexdigest(),
        tuple(sorted(cfg.items())))
    if key in _SESS:
        return _SESS[key]

    N, D, NC, GROUP = cfg["N"], cfg["D"], cfg["NCORES"], cfg["GROUP"]
    S, CH, BUCKET, NG, SP = _derive(cfg)
    pk = pack_graph(rows, cols, cfg)
    nc = build_nc(cfg, pk["KB1"], pk["KB2"])
    fn, in_names, out_names, out_avals, mesh = _make_exec(nc, NC)

    sh = NamedSharding(mesh, PartitionSpec("core"))
    iota = np.tile((0x4000 + np.arange(GROUP)).astype(np.uint16),
                   (128, 1)).view(ml_dtypes.bfloat16)
    statics = {
        "iotam": np.tile(iota, (NC, 1)),
        "gx1": pk["gx1"].reshape(NC * 128, -1),
        "dl1": pk["dl1"].reshape(NC * 128, -1),
        "gx2": pk["gx2"].reshape(NC * 128, -1),
        "dl2": pk["dl2"].reshape(NC * 128, -1),
        "degs": pk["degs"].reshape(NC * 3, -1),
    }
    dev = {k: jax.device_put(v, sh) for k, v in statics.items()}
    outbuf = jax.device_put(
        np.zeros((NC * S, D), np.float32), sh)

    sess = dict(fn=fn, in_names=in_names, out_names=out_names, dev=dev,
                outbuf=outbuf, cfg=cfg)
    _SESS[key] = sess
    return sess


def run(h_n, W_w, W_b, Wt_w, Wt_b, Ws_w, Ws_b, rows, cols, cfg):
    N, D, NC = cfg["N"], cfg["D"], cfg["NCORES"]
    sess = _get_session(rows, cols, cfg)

    h_np = np.ascontiguousarray(np.asarray(h_n), np.float32)
    wT3 = np.concatenate(
        [np.asarray(W_w).T, np.asarray(Wt_w).T, np.asarray(Ws_w).T],
        axis=1).astype(ml_dtypes.bfloat16)
    b3 = np.stack([np.asarray(W_b), np.asarray(Wt_b),
                   np.asarray(Ws_b)]).astype(ml_dtypes.bfloat16)

    per_call = {
        "h": h_np,
        "wT3": np.tile(wT3, (NC, 1)),
        "b3": np.tile(b3, (NC, 1)),
    }
    args = []
    for name in sess["in_names"]:
        if name in per_call:
            args.append(per_call[name])
        else:
            args.append(sess["dev"][name])
    args.append(sess["outbuf"])
    outs = sess["fn"](*args)
    o = outs[0]
    sess["outbuf"] = o
    return np.asarray(o)


def kernel(**inputs):
    return run(cfg=FULL_CFG, **inputs)
